# revision 5
# baseline (speedup 1.0000x reference)
"""Trainium2 Bass kernel for the MAB (moment-attention block) problem.

Math per batch element b (one NeuronCore each, B=8 == 8 cores, pure data
parallel, no collectives):

    q,k,v = split heads of Q[b], K[b] (V = K)
    S = q @ k^T / sqrt(512)            per head  [1024, 1024]
    A = softmax(S, axis=-1)            (no max-subtraction: |S| <~ 2)
    M  = A @ v, M2 = A @ v^2
    div = sqrt(max(M2 - M^2, eps))
    O_pre = [M | div]                  [1024, 1024]
    O = O_pre + relu(O_pre @ W_o^T + b_o)
    A1[b] = heads concat on last axis  [1024, 8192]

Layout strategy on-chip (per core):
  - Host pre-transposes Q^T, K^T (d-on-partition for the scores matmuls) and
    W_o^T with a feature permutation so that head h's [M_h ; div_h] features
    form one contiguous 128-partition chunk.
  - Scores are computed twice: once q-major (softmax + A1 output, ACT exp
    with fused accum row-sum), once k-major (feeds the A.V / A.V^2 matmul
    with stationary [V | V^2], which yields M-tilde/M2-tilde transposed =
    exactly the fc_o lhsT layout).
  - Softmax normalization: DVE reciprocal of the fused row-sums; A-tiles are
    scaled on GPSIMD (per-partition scalar); the transposed-side scaling uses
    an R matrix built by a tiny PE transpose + DRAM-broadcast DMA.
  - fc_o computed in natural [q, o] orientation; O_pre natural blocks are
    produced by PE half-transposes accumulated next to the DVE relu.
"""

import math

import numpy as np

H = 8          # heads
D = 64         # head dim
N = 1024       # sequence length (Nq == Nk)
DV = 512       # model dim
F = 2 * DV     # fc_o feature dim
NCORES = 8
SCALE = 1.0 / math.sqrt(DV)
EPS = 1e-6

_BUILT = {}


def _build(use_bias: bool):
    import concourse.bass as bass
    import concourse.mybir as mybir
    import concourse.tile as tile
    from concourse import bacc
    from concourse.masks import make_identity

    f32 = mybir.dt.float32
    Exp = mybir.ActivationFunctionType.Exp
    Sqrt = mybir.ActivationFunctionType.Sqrt

    nc = bacc.Bacc("TRN2", target_bir_lowering=False, debug=False,
                   num_devices=NCORES)

    QT = nc.dram_tensor("QT", [DV, N], f32, kind="ExternalInput")
    KT = nc.dram_tensor("KT", [DV, N], f32, kind="ExternalInput")
    KN = nc.dram_tensor("KN", [N, DV], f32, kind="ExternalInput")
    WOT = nc.dram_tensor("WOT", [F, F], f32, kind="ExternalInput")
    BO = nc.dram_tensor("BO", [1, F], f32, kind="ExternalInput") if use_bias else None
    O = nc.dram_tensor("O", [N, F], f32, kind="ExternalOutput")
    A1 = nc.dram_tensor("A1", [N, H * N], f32, kind="ExternalOutput")
    RD = nc.dram_tensor("RD", [H * 8, 128], f32)  # r rows staging (internal)

    with tile.TileContext(nc) as tc:
        with (
            tc.tile_pool(name="const", bufs=1) as cpool,
            tc.tile_pool(name="ppool", bufs=3) as ppool,
            tc.tile_pool(name="rpool", bufs=2) as rpool,
            tc.tile_pool(name="ptpool", bufs=1) as ptpool,
            tc.tile_pool(name="optpool", bufs=1) as optpool,
            tc.tile_pool(name="wpool", bufs=4) as wpool,
            tc.tile_pool(name="psA", bufs=2, space="PSUM") as psA,
            tc.tile_pool(name="psB", bufs=2, space="PSUM") as psB,
        ):
            # ---- constant loads -------------------------------------------
            QT_s = cpool.tile([128, 4 * N], f32)   # chunk j: QT rows j*128..
            KT_s = cpool.tile([128, 4 * N], f32)
            VA_s = cpool.tile([128, 8 * 1024], f32)  # chunk c: [KN_c | KN_c^2]
            WOT_s = cpool.tile([128, 8 * N], f32)  # chunk ic: WOT rows ic*128..
            IDT = cpool.tile([128, 128], f32)
            s_all = cpool.tile([128, 64], f32)
            r_all = cpool.tile([128, 64], f32)
            for j in range(4):
                nc.sync.dma_start(out=QT_s[:, j * N:(j + 1) * N],
                                  in_=QT[j * 128:(j + 1) * 128, :])
                nc.sync.dma_start(out=KT_s[:, j * N:(j + 1) * N],
                                  in_=KT[j * 128:(j + 1) * 128, :])
            for c in range(8):
                nc.sync.dma_start(out=VA_s[:, c * 1024:c * 1024 + 512],
                                  in_=KN[c * 128:(c + 1) * 128, :])
            for ic in range(8):
                nc.sync.dma_start(out=WOT_s[:, ic * N:(ic + 1) * N],
                                  in_=WOT[ic * 128:(ic + 1) * 128, :])
            make_identity(nc, IDT[:])
            if use_bias:
                B_s = cpool.tile([128, F], f32)
                bo_ap = BO[0:1, :]
                bcast = bass.AP(tensor=bo_ap.tensor, offset=bo_ap.offset,
                                ap=[[0, 128], [1, F]])
                nc.sync.dma_start(out=B_s[:], in_=bcast)
            for c in range(8):  # V^2 halves
                v = VA_s[:, c * 1024:c * 1024 + 512]
                nc.vector.tensor_mul(VA_s[:, c * 1024 + 512:c * 1024 + 1024], v, v)

            # [128, c, v, h, d] view of [KN_c | KN_c^2] chunks
            VA_view = VA_s[:].rearrange("p (c v h d) -> p c v h d",
                                        c=8, v=2, h=H)
            OPT = optpool.tile([128, 8 * N], f32)  # O_pre^T, permuted features

            # ---- per-head attention ---------------------------------------
            for h in range(H):
                hb = (h % 2) * 64      # partition base inside QT_s/KT_s chunk
                hc = (h // 2) * N      # free-dim base (chunk column offset)

                # Phase A: S = q k^T (q-major), exp + row-sums, A1 out.
                for qc in range(8):
                    idx = h * 8 + qc
                    ps = psA.tile([128, 1024], f32, tag="sa")
                    for j in range(2):
                        nc.tensor.matmul(
                            ps[:, j * 512:(j + 1) * 512],
                            lhsT=QT_s[hb:hb + 64, hc + qc * 128:hc + (qc + 1) * 128],
                            rhs=KT_s[hb:hb + 64, hc + j * 512:hc + (j + 1) * 512],
                            start=True, stop=True)
                    P = ppool.tile([128, 1024], f32, tag="p")
                    nc.scalar.activation(out=P[:], in_=ps[:], func=Exp,
                                         scale=SCALE,
                                         accum_out=s_all[:, idx:idx + 1])
                    nc.vector.reciprocal(out=r_all[:, idx:idx + 1],
                                         in_=s_all[:, idx:idx + 1])
                    nc.gpsimd.tensor_scalar_mul(P[:], P[:], r_all[:, idx:idx + 1])
                    nc.sync.dma_start(
                        out=A1[qc * 128:(qc + 1) * 128, h * N:(h + 1) * N],
                        in_=P[:])

                # Stage r (1/rowsum) into row layout and broadcast to R_h.
                prt = psB.tile([128, 1024], f32, tag="av")
                nc.tensor.transpose(out=prt[0:8, 0:128],
                                    in_=r_all[:, h * 8:h * 8 + 8],
                                    identity=IDT[:])
                rrow = wpool.tile([128, 1024], f32, tag="w")
                nc.vector.tensor_copy(rrow[0:8, 0:128], prt[0:8, 0:128])
                nc.sync.dma_start(out=RD[h * 8:h * 8 + 8, :], in_=rrow[0:8, 0:128])
                R_h = rpool.tile([128, 1024], f32, tag="r")
                rd_ap = RD[h * 8:h * 8 + 8, :]
                nc.sync.dma_start(out=R_h[:],
                                  in_=bass.AP(tensor=rd_ap.tensor,
                                              offset=rd_ap.offset,
                                              ap=[[0, 128], [1, 1024]]))

                # Per-head stationary [V_h | V_h^2] (contiguous for the PE):
                # VH cols = kc*128 + v*64 + d.
                VH = rpool.tile([128, 1024], f32, tag="vh")
                nc.gpsimd.tensor_copy(
                    VH[:].rearrange("p (c v d) -> p c v d", c=8, v=2),
                    VA_view[:, :, :, h, :])

                # Phase B: S^T (k-major), exp, fused [V|V^2] matmul.
                PT = ptpool.tile([128, 8 * 1024], f32, tag="pt")
                for kc in range(8):
                    ps = psA.tile([128, 1024], f32, tag="sa")
                    for j in range(2):
                        nc.tensor.matmul(
                            ps[:, j * 512:(j + 1) * 512],
                            lhsT=KT_s[hb:hb + 64, hc + kc * 128:hc + (kc + 1) * 128],
                            rhs=QT_s[hb:hb + 64, hc + j * 512:hc + (j + 1) * 512],
                            start=True, stop=True)
                    nc.scalar.activation(out=PT[:, kc * 1024:(kc + 1) * 1024],
                                         in_=ps[:], func=Exp, scale=SCALE)
                av = psB.tile([128, 1024], f32, tag="av")
                for kc in range(8):
                    for j in range(2):
                        nc.tensor.matmul(
                            av[:, j * 512:(j + 1) * 512],
                            lhsT=VH[:, kc * 128:(kc + 1) * 128],
                            rhs=PT[:, kc * 1024 + j * 512:kc * 1024 + (j + 1) * 512],
                            start=(kc == 0), stop=(kc == 7))

                # Epilogue: normalize, variance, write O_pre^T chunk h.
                mdst = OPT[0:64, h * N:(h + 1) * N]
                nc.vector.tensor_mul(mdst, av[0:64, :], R_h[0:64, :])
                mcopy = wpool.tile([128, 1024], f32, tag="w")
                nc.sync.dma_start(out=mcopy[64:128, :], in_=mdst)  # partition shift
                e2 = wpool.tile([128, 1024], f32, tag="w")
                nc.vector.tensor_mul(e2[64:128, :], av[64:128, :], R_h[64:128, :])
                nc.vector.tensor_mul(mcopy[64:128, :], mcopy[64:128, :],
                                     mcopy[64:128, :])
                nc.vector.tensor_sub(e2[64:128, :], e2[64:128, :], mcopy[64:128, :])
                nc.vector.tensor_scalar_max(OPT[64:128, h * N:(h + 1) * N],
                                            e2[64:128, :], EPS)

            # One sqrt pass over all div features (single ACT table switch).
            nc.scalar.activation(out=OPT[64:128, :], in_=OPT[64:128, :], func=Sqrt)

            # ---- fc_o + residual + output ---------------------------------
            for qc in range(8):
                fps = psA.tile([128, 1024], f32, tag="sa")
                for ic in range(8):
                    for j in range(2):
                        nc.tensor.matmul(
                            fps[:, j * 512:(j + 1) * 512],
                            lhsT=OPT[:, ic * N + qc * 128:ic * N + (qc + 1) * 128],
                            rhs=WOT_s[:, ic * N + j * 512:ic * N + (j + 1) * 512],
                            start=(ic == 0), stop=(ic == 7))
                rl = wpool.tile([128, 1024], f32, tag="w")
                if use_bias:
                    nc.vector.tensor_add(rl[:], fps[:], B_s[:])
                    nc.vector.tensor_scalar_max(rl[:], rl[:], 0.0)
                else:
                    nc.vector.tensor_scalar_max(rl[:], fps[:], 0.0)
                onat = psB.tile([128, 1024], f32, tag="av")
                for oc in range(8):
                    reg = oc if oc < 4 else oc - 4
                    pb = 0 if oc < 4 else 64
                    for half in range(2):
                        c = 2 * reg + half
                        nc.tensor.transpose(
                            out=onat[:, oc * 128 + half * 64:oc * 128 + half * 64 + 64],
                            in_=OPT[pb:pb + 64, c * N + qc * 128:c * N + (qc + 1) * 128],
                            identity=IDT[pb:pb + 64, pb:pb + 64])
                nc.vector.tensor_add(rl[:], onat[:], rl[:])
                nc.sync.dma_start(out=O[qc * 128:(qc + 1) * 128, :], in_=rl[:])

    nc.compile()
    return nc


def _get_built(use_bias: bool):
    if use_bias not in _BUILT:
        _BUILT[use_bias] = _build(use_bias)
    return _BUILT[use_bias]


def _feature_perm():
    perm = np.empty(F, dtype=np.int64)
    for h in range(H):
        perm[h * 128:h * 128 + 64] = np.arange(h * D, (h + 1) * D)
        perm[h * 128 + 64:h * 128 + 128] = DV + np.arange(h * D, (h + 1) * D)
    return perm


def kernel(Q, K, W_o, b_o):
    from concourse.bass_utils import run_bass_kernel_spmd

    Q = np.asarray(Q, dtype=np.float32)
    K = np.asarray(K, dtype=np.float32)
    W_o = np.asarray(W_o, dtype=np.float32)
    b_o = np.asarray(b_o, dtype=np.float32)
    B = Q.shape[0]
    assert B == NCORES and Q.shape == (B, N, DV) and K.shape == (B, N, DV)

    use_bias = bool(np.any(b_o != 0.0))
    nc = _get_built(use_bias)

    perm = _feature_perm()
    WOTP = np.ascontiguousarray(W_o.T[perm, :])
    in_maps = []
    for b in range(B):
        m = {
            "QT": np.ascontiguousarray(Q[b].T),
            "KT": np.ascontiguousarray(K[b].T),
            "KN": np.ascontiguousarray(K[b]),
            "WOT": WOTP,
        }
        if use_bias:
            m["BO"] = np.ascontiguousarray(b_o.reshape(1, F))
        in_maps.append(m)

    res = run_bass_kernel_spmd(nc, in_maps, list(range(NCORES)))
    O = np.stack([res.results[b]["O"] for b in range(B)]).astype(np.float32)
    A1 = np.stack([res.results[b]["A1"] for b in range(B)]).astype(np.float32)
    return (O, A1)


# revision 19
# speedup vs baseline: 1.0571x; 1.0571x over previous
"""Trainium2 Bass kernel for the MAB (moment-attention block) problem.

Math per batch element b (one NeuronCore each, B=8 == 8 cores, pure data
parallel, no collectives):

    q,k,v = split heads of Q[b], K[b] (V = K)
    S = q @ k^T / sqrt(512)            per head  [1024, 1024]
    A = softmax(S, axis=-1)            (no max-subtraction: |S| <~ 2)
    M  = A @ v, M2 = A @ v^2
    div = sqrt(max(M2 - M^2, eps))
    O_pre = [M | div]                  [1024, 1024]
    O = O_pre + relu(O_pre @ W_o^T + b_o)
    A1[b] = heads concat on last axis  [1024, 8192]

Layout strategy on-chip (per core):
  - Host pre-transposes Q^T, K^T (d-on-partition for the scores matmuls) and
    W_o^T with a feature permutation so that head h's [M_h ; div_h] features
    form one contiguous 128-partition chunk.
  - Scores are computed twice: once q-major (softmax + A1 output, ACT exp
    with fused accum row-sum), once k-major (feeds the A.V / A.V^2 matmul
    with stationary [V | V^2], which yields M-tilde/M2-tilde transposed =
    exactly the fc_o lhsT layout).  Matmuls run as float32r (full PE rate).
  - Softmax normalization: DVE reciprocal of the fused row-sums; A-tiles are
    scaled on GPSIMD (per-partition scalar); the transposed-side scaling uses
    an R matrix built by a tiny PE transpose + DRAM-broadcast DMA.
  - fc_o computed in natural [q, o] orientation; O_pre natural blocks are
    produced by PE half-transposes accumulated next to the DVE relu.
  - The head loop is software-pipelined (phase-A(h+1) emitted between
    B-scores(h) and A.V(h)) so the PE keeps feeding ACT, the pacing engine.
"""

import math

import numpy as np

H = 8          # heads
D = 64         # head dim
N = 1024       # sequence length (Nq == Nk)
DV = 512       # model dim
F = 2 * DV     # fc_o feature dim
NCORES = 8
SCALE = 1.0 / math.sqrt(DV)
EPS = 1e-6

_BUILT = {}


def _build(use_bias: bool):
    import concourse.bass as bass
    import concourse.mybir as mybir
    import concourse.tile as tile
    from concourse import bacc
    from concourse.masks import make_identity

    f32 = mybir.dt.float32
    f32r = mybir.dt.float32r
    rr = lambda ap: ap.bitcast(f32r)
    Exp = mybir.ActivationFunctionType.Exp
    Sqrt = mybir.ActivationFunctionType.Sqrt

    nc = bacc.Bacc("TRN2", target_bir_lowering=False, debug=False,
                   num_devices=NCORES)

    QT = nc.dram_tensor("QT", [DV, N], f32, kind="ExternalInput")
    KT = nc.dram_tensor("KT", [DV, N], f32, kind="ExternalInput")
    KN = nc.dram_tensor("KN", [N, DV], f32, kind="ExternalInput")
    WOT = nc.dram_tensor("WOT", [F, F], f32, kind="ExternalInput")
    BO = nc.dram_tensor("BO", [1, F], f32, kind="ExternalInput") if use_bias else None
    O = nc.dram_tensor("O", [N, F], f32, kind="ExternalOutput")
    A1 = nc.dram_tensor("A1", [N, H * N], f32, kind="ExternalOutput")
    RD = nc.dram_tensor("RD", [H * 8, 128], f32)  # r rows staging (internal)

    with tile.TileContext(nc) as tc:
        with (
            tc.tile_pool(name="const", bufs=1) as cpool,
            tc.tile_pool(name="ppool", bufs=6) as ppool,
            tc.tile_pool(name="rpool", bufs=2) as rpool,
            tc.tile_pool(name="ptpool", bufs=1) as ptpool,
            tc.tile_pool(name="optpool", bufs=1) as optpool,
            tc.tile_pool(name="wpool", bufs=4) as wpool,
            tc.tile_pool(name="spool", bufs=4) as spool,
            tc.tile_pool(name="psA", bufs=2, space="PSUM") as psA,
            tc.tile_pool(name="psB", bufs=2, space="PSUM") as psB,
        ):
            # ---- constant loads -------------------------------------------
            QT_s = cpool.tile([128, 4 * N], f32)   # chunk j: QT rows j*128..
            KT_s = cpool.tile([128, 4 * N], f32)
            KN_s = cpool.tile([128, 4096], f32)    # chunk c: KN rows c*128..
            WOT_s = cpool.tile([128, 8 * N], f32)  # chunk ic: WOT rows ic*128..
            IDT = cpool.tile([128, 128], f32)
            s_all = cpool.tile([128, 64], f32)
            r_all = cpool.tile([128, 64], f32)
            nc.sync.dma_start(out=rr(QT_s[:, 0:N]), in_=rr(QT[0:128, :]))
            nc.sync.dma_start(out=rr(KT_s[:, 0:N]), in_=rr(KT[0:128, :]))

            # Deferred loads, drip-fed into the SP DMA queue so they never
            # starve the A1-store pipeline (which recycles exp output tiles).
            fillers = []
            for c in range(8):
                fillers.append((KN_s[:, c * 512:(c + 1) * 512],
                                KN[c * 128:(c + 1) * 128, :]))
            for j in range(1, 4):
                fillers.append((rr(QT_s[:, j * N:(j + 1) * N]),
                                rr(QT[j * 128:(j + 1) * 128, :])))
                fillers.append((rr(KT_s[:, j * N:(j + 1) * N]),
                                rr(KT[j * 128:(j + 1) * 128, :])))
            for ic in range(8):
                fillers.append((rr(WOT_s[:, ic * N:(ic + 1) * N]),
                                rr(WOT[ic * 128:(ic + 1) * 128, :])))
            fillers.reverse()

            def emit_fill(n):
                for _ in range(n):
                    if fillers:
                        dst, srcap = fillers.pop()
                        nc.sync.dma_start(out=dst, in_=srcap)

            make_identity(nc, IDT[:])
            if use_bias:
                B_s = cpool.tile([128, F], f32)
                bo_ap = BO[0:1, :]
                bcast = bass.AP(tensor=bo_ap.tensor, offset=bo_ap.offset,
                                ap=[[0, 128], [1, F]])
                nc.sync.dma_start(out=B_s[:], in_=bcast)

            KN_view = KN_s[:].rearrange("p (c h d) -> p c h d", c=8, h=H)
            OPT = optpool.tile([128, 8 * N], f32)  # O_pre^T, permuted features

            def emit_a_mm(h, qc):
                hb = (h % 2) * 64
                hc = (h // 2) * N
                ps = psA.tile([128, 1024], f32, tag="sa")
                for j in range(2):
                    nc.tensor.matmul(
                        ps[:, j * 512:(j + 1) * 512],
                        lhsT=rr(QT_s[hb:hb + 64, hc + qc * 128:hc + (qc + 1) * 128]),
                        rhs=rr(KT_s[hb:hb + 64, hc + j * 512:hc + (j + 1) * 512]),
                        start=True, stop=True)
                return ps

            def emit_a_tail(h, qc, ps, st):
                """exp(+accum) -> (per pair) recip -> normalize -> A1 store."""
                P = ppool.tile([128, 1024], f32, tag="p")
                if qc % 2 == 0:
                    st["s"] = spool.tile([128, 2], f32, tag="s", name="s_t")
                nc.scalar.activation(out=P[:], in_=ps[:], func=Exp, scale=SCALE,
                                     accum_out=st["s"][:, qc % 2:qc % 2 + 1])
                st[qc] = P
                if qc % 2 == 1:
                    i0 = h * 8 + qc - 1
                    nc.vector.reciprocal(out=r_all[:, i0:i0 + 2], in_=st["s"][:, 0:2])
                    for q2 in (qc - 1, qc):
                        i2 = h * 8 + q2
                        P2 = st.pop(q2)
                        nc.gpsimd.tensor_scalar_mul(P2[:], P2[:], r_all[:, i2:i2 + 1])
                        nc.sync.dma_start(
                            out=A1[q2 * 128:(q2 + 1) * 128, h * N:(h + 1) * N],
                            in_=P2[:])
                    emit_fill(2)

            def emit_rstage(h):
                """r (1/rowsum) into row layout, broadcast to R_h; build VH."""
                prt = psB.tile([128, 1024], f32, tag="av")
                nc.tensor.transpose(out=prt[0:8, 0:128],
                                    in_=r_all[:, h * 8:h * 8 + 8],
                                    identity=IDT[:])
                rrow = wpool.tile([128, 1024], f32, tag="w")
                nc.vector.tensor_copy(rrow[0:8, 0:128], prt[0:8, 0:128])
                nc.sync.dma_start(out=RD[h * 8:h * 8 + 8, :], in_=rrow[0:8, 0:128])
                R_h = rpool.tile([128, 1024], f32, tag="r")
                rd_ap = RD[h * 8:h * 8 + 8, :]
                nc.sync.dma_start(out=R_h[:],
                                    in_=bass.AP(tensor=rd_ap.tensor,
                                                offset=rd_ap.offset,
                                                ap=[[0, 128], [1, 1024]]))
                VH = rpool.tile([128, 1024], f32, tag="vh")
                VH_view = VH[:].rearrange("p (c v d) -> p c v d", c=8, v=2)
                nc.gpsimd.tensor_copy(rr(VH_view[:, :, 0, :]), KN_view[:, :, h, :])
                nc.vector.tensor_mul(rr(VH_view[:, :, 1, :]), KN_view[:, :, h, :],
                                     KN_view[:, :, h, :])
                return R_h, VH

            def emit_b_mm(h, kc, PT):
                hb = (h % 2) * 64
                hc = (h // 2) * N
                ps = psA.tile([128, 1024], f32, tag="sa")
                for j in range(2):
                    nc.tensor.matmul(
                        ps[:, j * 512:(j + 1) * 512],
                        lhsT=rr(KT_s[hb:hb + 64, hc + kc * 128:hc + (kc + 1) * 128]),
                        rhs=rr(QT_s[hb:hb + 64, hc + j * 512:hc + (j + 1) * 512]),
                        start=True, stop=True)
                nc.scalar.activation(out=rr(PT[:, kc * 1024:(kc + 1) * 1024]),
                                     in_=ps[:], func=Exp, scale=SCALE)

            def emit_av_mm(h, kc, av, VH, PT):
                for j in range(2):
                    nc.tensor.matmul(
                        av[:, j * 512:(j + 1) * 512],
                        lhsT=rr(VH[:, kc * 128:(kc + 1) * 128]),
                        rhs=rr(PT[:, kc * 1024 + j * 512:kc * 1024 + (j + 1) * 512]),
                        start=(kc == 0), stop=(kc == 7),
                        skip_group_check=True)

            def emit_epilogue(h, av, R_h):
                """Normalize, variance; write O_pre^T chunk h."""
                mdst = OPT[0:64, h * N:(h + 1) * N]
                nc.vector.tensor_mul(rr(mdst), av[0:64, :], R_h[0:64, :])
                mcopy = wpool.tile([128, 1024], f32, tag="w")
                nc.sync.dma_start(out=mcopy[64:128, :], in_=mdst)  # part shift
                e2 = wpool.tile([128, 1024], f32, tag="w")
                nc.vector.tensor_mul(e2[64:128, :], av[64:128, :], R_h[64:128, :])
                nc.vector.tensor_mul(mcopy[64:128, :], mcopy[64:128, :],
                                     mcopy[64:128, :])
                nc.vector.tensor_sub(e2[64:128, :], e2[64:128, :], mcopy[64:128, :])
                nc.vector.tensor_scalar_max(rr(OPT[64:128, h * N:(h + 1) * N]),
                                             e2[64:128, :], EPS)

            # ---- software-pipelined head loop -----------------------------
            # Per step: B-scores(h)+exp, A-scores(h+1)+exp, AV(h, step-2).
            # AV matmuls trail the exp stream by 2 steps so the PE never
            # waits on ACT, and ACT never waits on the PE.
            st0 = {}
            for qc in range(8):
                emit_a_tail(0, qc, emit_a_mm(0, qc), st0)
            for h in range(H):
                R_h, VH = emit_rstage(h)
                PT = ptpool.tile([128, 8 * 1024], f32, tag="pt")
                av = psB.tile([128, 1024], f32, tag="av")
                stn = {}
                for step in range(8):
                    emit_b_mm(h, step, PT)
                    if h + 1 < H:
                        emit_a_tail(h + 1, step, emit_a_mm(h + 1, step), stn)
                    if step >= 2:
                        emit_av_mm(h, step - 2, av, VH, PT)
                if h == H - 1:
                    # All exps done: sqrt heads 0..6 now (one table switch),
                    # overlapping the last head's AV + epilogue.
                    for h2 in range(H - 1):
                        nc.scalar.activation(out=rr(OPT[64:128, h2 * N:(h2 + 1) * N]),
                                             in_=OPT[64:128, h2 * N:(h2 + 1) * N],
                                             func=Sqrt)
                for kc in (6, 7):
                    emit_av_mm(h, kc, av, VH, PT)
                emit_epilogue(h, av, R_h)

            h2 = H - 1
            nc.scalar.activation(out=rr(OPT[64:128, h2 * N:(h2 + 1) * N]),
                                 in_=OPT[64:128, h2 * N:(h2 + 1) * N], func=Sqrt)

            # ---- fc_o + residual + output ---------------------------------
            for qc in range(8):
                fps = psA.tile([128, 1024], f32, tag="sa")
                for ic in range(8):
                    for j in range(2):
                        nc.tensor.matmul(
                            fps[:, j * 512:(j + 1) * 512],
                            lhsT=rr(OPT[:, ic * N + qc * 128:ic * N + (qc + 1) * 128]),
                            rhs=rr(WOT_s[:, ic * N + j * 512:ic * N + (j + 1) * 512]),
                            start=(ic == 0), stop=(ic == 7))
                rl = wpool.tile([128, 1024], f32, tag="w")
                if use_bias:
                    nc.vector.tensor_add(rl[:], fps[:], B_s[:])
                    nc.vector.tensor_scalar_max(rl[:], rl[:], 0.0)
                else:
                    nc.vector.tensor_scalar_max(rl[:], fps[:], 0.0)
                onat = psB.tile([128, 1024], f32, tag="av")
                for oc in range(8):
                    reg = oc if oc < 4 else oc - 4
                    pb = 0 if oc < 4 else 64
                    for half in range(2):
                        c = 2 * reg + half
                        nc.tensor.transpose(
                            out=onat[:, oc * 128 + half * 64:oc * 128 + half * 64 + 64],
                            in_=OPT[pb:pb + 64, c * N + qc * 128:c * N + (qc + 1) * 128],
                            identity=IDT[pb:pb + 64, pb:pb + 64])
                nc.vector.tensor_add(rl[:], onat[:], rl[:])
                nc.sync.dma_start(out=O[qc * 128:(qc + 1) * 128, :], in_=rl[:])

    nc.compile()
    return nc


def _get_built(use_bias: bool):
    if use_bias not in _BUILT:
        _BUILT[use_bias] = _build(use_bias)
    return _BUILT[use_bias]


def _feature_perm():
    perm = np.empty(F, dtype=np.int64)
    for h in range(H):
        perm[h * 128:h * 128 + 64] = np.arange(h * D, (h + 1) * D)
        perm[h * 128 + 64:h * 128 + 128] = DV + np.arange(h * D, (h + 1) * D)
    return perm


def kernel(Q, K, W_o, b_o):
    from concourse.bass_utils import run_bass_kernel_spmd

    Q = np.asarray(Q, dtype=np.float32)
    K = np.asarray(K, dtype=np.float32)
    W_o = np.asarray(W_o, dtype=np.float32)
    b_o = np.asarray(b_o, dtype=np.float32)
    B = Q.shape[0]
    assert B == NCORES and Q.shape == (B, N, DV) and K.shape == (B, N, DV)

    use_bias = bool(np.any(b_o != 0.0))
    nc = _get_built(use_bias)

    perm = _feature_perm()
    WOTP = np.ascontiguousarray(W_o.T[perm, :])
    in_maps = []
    for b in range(B):
        m = {
            "QT": np.ascontiguousarray(Q[b].T),
            "KT": np.ascontiguousarray(K[b].T),
            "KN": np.ascontiguousarray(K[b]),
            "WOT": WOTP,
        }
        if use_bias:
            m["BO"] = np.ascontiguousarray(b_o.reshape(1, F))
        in_maps.append(m)

    res = run_bass_kernel_spmd(nc, in_maps, list(range(NCORES)))
    O = np.stack([res.results[b]["O"] for b in range(B)]).astype(np.float32)
    A1 = np.stack([res.results[b]["A1"] for b in range(B)]).astype(np.float32)
    return (O, A1)
